# revision 5
# baseline (speedup 1.0000x reference)
"""Expert-parallel MoE FFN kernel for 8 Trainium2 NeuronCores.

Problem: dense-MoE (every expert runs on all tokens) with top-2 routing:
    h  = gelu(x @ W1[e] + b1[e])            (E,B,S,H)
    eo = h @ W2[e] + b2[e]                  (E,B,S,D)
    out = einsum('ebsd,bse->bsd', eo, masked_scores)
    counts[e] = number of tokens with expert e in top-2

Sharding: core e owns expert e (weights + intermediates); the weighted
combine over E is a ReduceScatter(add) across the 8 cores; each core
returns a 128-row shard of the [D, T] output which the host concatenates
and transposes.

Device layout (per core), everything feature-major ([feature, token]):
    mm1: lhsT = W1 tiles [128k, 128m], rhs = xT tiles [128k, 512t]
         -> psum hT [128h, 512t]; ACT gelu(+b1 per-partition) -> SBUF bf16
    mm2: lhsT = W2 tiles, rhs = hT -> psum eoT [128d, 512t];
         DVE (+b2, *masked_score[t]) in PSUM; DMA PSUM -> DRAM bounce [D, T]
    ReduceScatter over cores on the bounce; shard r = rows [128r, 128(r+1)).

Weights are cast to bf16 on the host; accumulation is fp32 in PSUM; the
combine path (bias add, score scaling, reduce) is fp32.
"""

import sys

if "/opt/trn_rl_repo" not in sys.path:
    sys.path.insert(0, "/opt/trn_rl_repo")

import ml_dtypes
import numpy as np

from concourse import bass, mybir, tile
from concourse.bass_utils import run_bass_kernel_spmd

B, S, D, H, E = 4, 1024, 1024, 4096, 8
TOPK = 2
T = B * S                       # 4096 tokens
P = 128
NCORES = 8
TC = 512                        # token chunk (matmul moving free dim)
NCHUNK = T // TC                # 8
KO1 = D // P                    # 8  k-subtiles of mm1
M1 = H // P                     # 32 m-tiles of mm1 (hT partition tiles)
KO2 = H // P                    # 32 k-subtiles of mm2
M2 = D // P                     # 8  m-tiles of mm2 (eoT partition tiles)

BF16 = mybir.dt.bfloat16
F32 = mybir.dt.float32


def split_multi_waits(nc):
    """This walrus build only accepts one semaphore wait per instruction.
    Move extra waits onto same-engine nop instructions inserted just before
    the owning instruction (the engine queue blocks on each in turn)."""
    import bass_rust

    uid = 0
    for fn in nc.m.functions:
        for bb in fn.blocks:
            insts = bb.instructions
            i = 0
            while i < len(insts):
                inst = insts[i]
                si = inst.sync_info
                if si is not None and len(si.on_wait) > 1:
                    extras = list(si.on_wait[1:])
                    del si.on_wait[1:]
                    for w in extras:
                        nop = bass_rust.InstNoOp(
                            name=f"I-waitsplit-{uid}", ins=[], outs=[]
                        )
                        uid += 1
                        nop.engine = inst.engine
                        nop.sync_info = mybir.SyncInfo(on_wait=[w], on_update=[])
                        insts.insert(i, nop)
                        i += 1
                i += 1


_NC_CACHE = None


def build_nc():
    global _NC_CACHE
    if _NC_CACHE is not None:
        return _NC_CACHE

    nc = bass.Bass(num_devices=NCORES)

    xt_d = nc.dram_tensor("xt", [NCHUNK, P, KO1, TC], BF16, kind="ExternalInput")
    w1_d = nc.dram_tensor("w1", [M1, P, KO1, P], BF16, kind="ExternalInput")
    w2_d = nc.dram_tensor("w2", [M2, P, KO2, P], BF16, kind="ExternalInput")
    b1_d = nc.dram_tensor("b1", [P, M1], F32, kind="ExternalInput")
    b2_d = nc.dram_tensor("b2", [P, M2], F32, kind="ExternalInput")
    msb_d = nc.dram_tensor("msb", [P, T], F32, kind="ExternalInput")
    out_d = nc.dram_tensor("out_shard", [P, T], F32, kind="ExternalOutput")

    with tile.TileContext(nc) as tc:
        with (
            tc.tile_pool(name="wpool", bufs=1) as wpool,
            tc.tile_pool(name="cpool", bufs=1) as cpool,
            tc.tile_pool(name="xpool", bufs=2) as xpool,
            tc.tile_pool(name="mspool", bufs=2) as mspool,
            tc.tile_pool(name="htpool", bufs=1) as htpool,
            tc.tile_pool(name="outpool", bufs=3) as outpool,
            tc.tile_pool(name="ps1", bufs=4, space="PSUM") as ps1,
            tc.tile_pool(name="ps2", bufs=4, space="PSUM") as ps2,
            tc.tile_pool(name="dram", bufs=1, space="DRAM") as dram,
        ):
            # resident weights, loaded as strips so compute can start early
            w1_sb = []
            for m in range(M1):
                w1_strip = wpool.tile([P, KO1, P], BF16, name=f"w1_{m}", tag=f"w1_{m}")
                nc.sync.dma_start(w1_strip[:], w1_d[m])
                w1_sb.append(w1_strip)
            w2_sb = []
            for n in range(M2):
                w2_strip = wpool.tile([P, KO2, P], BF16, name=f"w2_{n}", tag=f"w2_{n}")
                nc.sync.dma_start(w2_strip[:], w2_d[n])
                w2_sb.append(w2_strip)

            b1_sb = cpool.tile([P, M1], F32)
            nc.sync.dma_start(b1_sb[:], b1_d[:])
            b2_sb = cpool.tile([P, M2], F32)
            nc.sync.dma_start(b2_sb[:], b2_d[:])

            bounce = dram.tile([D, T], F32)
            rs_out = dram.tile([P, T], F32)

            for c in range(NCHUNK):
                xt_sb = xpool.tile([P, KO1, TC], BF16, tag="xt")
                nc.sync.dma_start(xt_sb[:], xt_d[c])
                ms_sb = mspool.tile([P, TC], F32, tag="ms")
                nc.sync.dma_start(ms_sb[:], msb_d[:, c * TC : (c + 1) * TC])

                ht_sb = htpool.tile([P, M1, TC], BF16, tag="ht")
                for m in range(M1):
                    psum_h = ps1.tile([P, TC], F32, tag="psh")
                    for k in range(KO1):
                        nc.tensor.matmul(
                            psum_h[:],
                            w1_sb[m][:, k, :],
                            xt_sb[:, k, :],
                            start=(k == 0),
                            stop=(k == KO1 - 1),
                        )
                    nc.scalar.activation(
                        ht_sb[:, m, :],
                        psum_h[:],
                        mybir.ActivationFunctionType.Gelu,
                        bias=b1_sb[:, m : m + 1],
                    )

                for n in range(M2):
                    psum_o = ps2.tile([P, TC], F32, tag="pso")
                    for k in range(KO2):
                        nc.tensor.matmul(
                            psum_o[:],
                            w2_sb[n][:, k, :],
                            ht_sb[:, k, :],
                            start=(k == 0),
                            stop=(k == KO2 - 1),
                        )
                    out_sb = outpool.tile([P, TC], F32, tag="out")
                    nc.vector.tensor_scalar_add(out_sb[:], psum_o[:], b2_sb[:, n : n + 1])
                    nc.vector.tensor_mul(out_sb[:], out_sb[:], ms_sb[:])
                    nc.sync.dma_start(
                        bounce[n * P : (n + 1) * P, c * TC : (c + 1) * TC], out_sb[:]
                    )

            nc.gpsimd.collective_compute(
                "ReduceScatter",
                mybir.AluOpType.add,
                replica_groups=[list(range(NCORES))],
                ins=[bounce[:]],
                outs=[rs_out[:]],
            )
            nc.sync.dma_start(out_d[:], rs_out[:])

    split_multi_waits(nc)
    _NC_CACHE = nc
    return nc


def routing_host(routing_scores):
    """Replicate the reference's top-k mask/scale in numpy fp32."""
    scores = np.asarray(routing_scores, dtype=np.float32)
    order = np.argsort(-scores, axis=-1, kind="stable")
    topk_idx = order[..., :TOPK]                                   # (B,S,K)
    vals = np.take_along_axis(scores, topk_idx, axis=-1)           # (B,S,K)
    mask = np.zeros(scores.shape, dtype=bool)
    np.put_along_axis(mask, topk_idx, True, axis=-1)
    scale = np.float32(1.0) / (
        vals.sum(axis=-1, keepdims=True, dtype=np.float32) + np.float32(1e-8)
    )
    masked_scores = (scale * scores) * mask                        # (B,S,E) f32
    counts = mask.sum(axis=0).sum(axis=0).astype(np.int32)         # (E,)
    return masked_scores.astype(np.float32), counts


def make_in_maps(x, routing_scores, W1, b1, W2, b2):
    x = np.asarray(x, dtype=np.float32)
    W1 = np.asarray(W1, dtype=np.float32)
    b1 = np.asarray(b1, dtype=np.float32)
    W2 = np.asarray(W2, dtype=np.float32)
    b2 = np.asarray(b2, dtype=np.float32)

    masked_scores, counts = routing_host(routing_scores)

    # xT in chunk-major layout [c, p, ko, t_local]
    x_tok = x.reshape(T, D).astype(ml_dtypes.bfloat16)
    xt = np.ascontiguousarray(
        x_tok.reshape(NCHUNK, TC, KO1, P).transpose(0, 3, 2, 1)
    )

    in_maps = []
    for e in range(NCORES):
        w1e = np.ascontiguousarray(
            W1[e].astype(ml_dtypes.bfloat16).reshape(KO1, P, M1, P).transpose(2, 1, 0, 3)
        )
        w2e = np.ascontiguousarray(
            W2[e].astype(ml_dtypes.bfloat16).reshape(KO2, P, M2, P).transpose(2, 1, 0, 3)
        )
        b1e = np.ascontiguousarray(b1[e].reshape(M1, P).T)
        b2e = np.ascontiguousarray(b2[e].reshape(M2, P).T)
        mse = np.ascontiguousarray(
            np.broadcast_to(
                masked_scores[:, :, e].reshape(1, T), (P, T)
            )
        ).astype(np.float32)
        in_maps.append(
            {"xt": xt, "w1": w1e, "w2": w2e, "b1": b1e, "b2": b2e, "msb": mse}
        )
    return in_maps, counts


def gather_output(results):
    outT = np.concatenate(
        [results[r]["out_shard"] for r in range(NCORES)], axis=0
    )                                                              # [D, T]
    return np.ascontiguousarray(outT.T).reshape(B, S, D)


def kernel(x, routing_scores, W1, b1, W2, b2):
    nc = build_nc()
    in_maps, counts = make_in_maps(x, routing_scores, W1, b1, W2, b2)
    res = run_bass_kernel_spmd(nc, in_maps, core_ids=list(range(NCORES)))
    final = gather_output(res.results)
    return final, counts


# revision 9
# speedup vs baseline: 14.0738x; 14.0738x over previous
"""Expert-parallel MoE FFN kernel for 8 Trainium2 NeuronCores.

Problem: dense-MoE (every expert runs on all tokens) with top-2 routing:
    h  = gelu(x @ W1[e] + b1[e])            (E,B,S,H)
    eo = h @ W2[e] + b2[e]                  (E,B,S,D)
    out = einsum('ebsd,bse->bsd', eo, masked_scores)
    counts[e] = number of tokens with expert e in top-2

Sharding: core e owns expert e (weights + intermediates); the weighted
combine over E is a ReduceScatter(add) across the 8 cores; each core
returns a 128-row shard of the [D, T] output which the host concatenates
and transposes.

Device layout (per core), everything feature-major ([feature, token]):
    mm1: lhsT = W1 tiles [128k, 128m], rhs = xT tiles [128k, 512t]
         -> psum hT [128h, 512t]; ACT gelu(+b1 per-partition) -> SBUF bf16
    mm2: lhsT = W2 tiles, rhs = hT -> psum eoT [128d, 512t];
         DVE (+b2, *masked_score[t]) in PSUM; DMA PSUM -> DRAM bounce [D, T]
    ReduceScatter over cores on the bounce; shard r = rows [128r, 128(r+1)).

Weights are cast to bf16 on the host; accumulation is fp32 in PSUM; the
combine path (bias add, score scaling, reduce) is fp32.
"""

import sys

if "/opt/trn_rl_repo" not in sys.path:
    sys.path.insert(0, "/opt/trn_rl_repo")

import ml_dtypes
import numpy as np

from concourse import bass, mybir, tile
from concourse.bass_utils import run_bass_kernel_spmd

B, S, D, H, E = 4, 1024, 1024, 4096, 8
TOPK = 2
T = B * S                       # 4096 tokens
P = 128
NCORES = 8
TC = 512                        # token chunk (matmul moving free dim)
NCHUNK = T // TC                # 8
KO1 = D // P                    # 8  k-subtiles of mm1
M1 = H // P                     # 32 m-tiles of mm1 (hT partition tiles)
KO2 = H // P                    # 32 k-subtiles of mm2
M2 = D // P                     # 8  m-tiles of mm2 (eoT partition tiles)

BF16 = mybir.dt.bfloat16
F32 = mybir.dt.float32


def split_multi_waits(nc):
    """This walrus build only accepts one semaphore wait per instruction.
    Move extra waits onto same-engine nop instructions inserted just before
    the owning instruction (the engine queue blocks on each in turn)."""
    import bass_rust

    uid = 0
    for fn in nc.m.functions:
        for bb in fn.blocks:
            insts = bb.instructions
            i = 0
            while i < len(insts):
                inst = insts[i]
                si = inst.sync_info
                if si is not None and len(si.on_wait) > 1:
                    extras = list(si.on_wait[1:])
                    del si.on_wait[1:]
                    for w in extras:
                        nop = bass_rust.InstNoOp(
                            name=f"I-waitsplit-{uid}", ins=[], outs=[]
                        )
                        uid += 1
                        nop.engine = inst.engine
                        nop.sync_info = mybir.SyncInfo(on_wait=[w], on_update=[])
                        insts.insert(i, nop)
                        i += 1
                i += 1


_NC_CACHE = {}


def build_nc(reps=1):
    """Build the SPMD program. ``reps`` repeats the whole compute+reduce body
    (same inputs/outputs) — used by the test harness to measure device time
    as a slope across reps, cancelling the fixed dispatch overhead."""
    if reps in _NC_CACHE:
        return _NC_CACHE[reps]

    nc = bass.Bass(num_devices=NCORES)

    xt_d = nc.dram_tensor("xt", [NCHUNK, P, KO1, TC], BF16, kind="ExternalInput")
    w1_d = nc.dram_tensor("w1", [M1, P, KO1, P], BF16, kind="ExternalInput")
    w2_d = nc.dram_tensor("w2", [M2, P, KO2, P], BF16, kind="ExternalInput")
    b1_d = nc.dram_tensor("b1", [P, M1], F32, kind="ExternalInput")
    b2_d = nc.dram_tensor("b2", [P, M2], F32, kind="ExternalInput")
    msb_d = nc.dram_tensor("msb", [P, T], F32, kind="ExternalInput")
    out_d = nc.dram_tensor("out_shard", [P, T], F32, kind="ExternalOutput")

    with tile.TileContext(nc) as tc:
        with (
            tc.tile_pool(name="wpool", bufs=1) as wpool,
            tc.tile_pool(name="cpool", bufs=1) as cpool,
            tc.tile_pool(name="xpool", bufs=2) as xpool,
            tc.tile_pool(name="mspool", bufs=2) as mspool,
            tc.tile_pool(name="htpool", bufs=1) as htpool,
            tc.tile_pool(name="outpool", bufs=3) as outpool,
            tc.tile_pool(name="ps1", bufs=4, space="PSUM") as ps1,
            tc.tile_pool(name="ps2", bufs=4, space="PSUM") as ps2,
            tc.tile_pool(name="dram", bufs=1, space="DRAM") as dram,
        ):
            # resident weights, loaded as strips so compute can start early
            w1_sb = []
            for m in range(M1):
                w1_strip = wpool.tile([P, KO1, P], BF16, name=f"w1_{m}", tag=f"w1_{m}")
                nc.sync.dma_start(w1_strip[:], w1_d[m])
                w1_sb.append(w1_strip)
            w2_sb = []
            for n in range(M2):
                w2_strip = wpool.tile([P, KO2, P], BF16, name=f"w2_{n}", tag=f"w2_{n}")
                nc.sync.dma_start(w2_strip[:], w2_d[n])
                w2_sb.append(w2_strip)

            b1_sb = cpool.tile([P, M1], F32)
            nc.sync.dma_start(b1_sb[:], b1_d[:])
            b2_sb = cpool.tile([P, M2], F32)
            nc.sync.dma_start(b2_sb[:], b2_d[:])

            bounce = dram.tile([D, T], F32)
            rs_out = dram.tile([P, T], F32)

            for _rep in range(reps):
              for c in range(NCHUNK):
                xt_sb = xpool.tile([P, KO1, TC], BF16, tag="xt")
                nc.sync.dma_start(xt_sb[:], xt_d[c])
                ms_sb = mspool.tile([P, TC], F32, tag="ms")
                nc.sync.dma_start(ms_sb[:], msb_d[:, c * TC : (c + 1) * TC])

                ht_sb = htpool.tile([P, M1, TC], BF16, tag="ht")
                for m in range(M1):
                    psum_h = ps1.tile([P, TC], F32, tag="psh")
                    for k in range(KO1):
                        nc.tensor.matmul(
                            psum_h[:],
                            w1_sb[m][:, k, :],
                            xt_sb[:, k, :],
                            start=(k == 0),
                            stop=(k == KO1 - 1),
                        )
                    nc.scalar.activation(
                        ht_sb[:, m, :],
                        psum_h[:],
                        mybir.ActivationFunctionType.Gelu,
                        bias=b1_sb[:, m : m + 1],
                    )

                for n in range(M2):
                    psum_o = ps2.tile([P, TC], F32, tag="pso")
                    for k in range(KO2):
                        nc.tensor.matmul(
                            psum_o[:],
                            w2_sb[n][:, k, :],
                            ht_sb[:, k, :],
                            start=(k == 0),
                            stop=(k == KO2 - 1),
                        )
                    out_sb = outpool.tile([P, TC], F32, tag="out")
                    nc.vector.tensor_scalar_add(out_sb[:], psum_o[:], b2_sb[:, n : n + 1])
                    nc.vector.tensor_mul(out_sb[:], out_sb[:], ms_sb[:])
                    nc.sync.dma_start(
                        bounce[n * P : (n + 1) * P, c * TC : (c + 1) * TC], out_sb[:]
                    )

              nc.gpsimd.collective_compute(
                  "ReduceScatter",
                  mybir.AluOpType.add,
                  replica_groups=[list(range(NCORES))],
                  ins=[bounce[:]],
                  outs=[rs_out[:]],
              )
              nc.sync.dma_start(out_d[:], rs_out[:])

    split_multi_waits(nc)
    _NC_CACHE[reps] = nc
    return nc


def routing_host(routing_scores):
    """Replicate the reference's top-k mask/scale in numpy fp32."""
    scores = np.asarray(routing_scores, dtype=np.float32)
    order = np.argsort(-scores, axis=-1, kind="stable")
    topk_idx = order[..., :TOPK]                                   # (B,S,K)
    vals = np.take_along_axis(scores, topk_idx, axis=-1)           # (B,S,K)
    mask = np.zeros(scores.shape, dtype=bool)
    np.put_along_axis(mask, topk_idx, True, axis=-1)
    scale = np.float32(1.0) / (
        vals.sum(axis=-1, keepdims=True, dtype=np.float32) + np.float32(1e-8)
    )
    masked_scores = (scale * scores) * mask                        # (B,S,E) f32
    counts = mask.sum(axis=0).sum(axis=0).astype(np.int32)         # (E,)
    return masked_scores.astype(np.float32), counts


def make_in_maps(x, routing_scores, W1, b1, W2, b2):
    x = np.asarray(x, dtype=np.float32)
    W1 = np.asarray(W1, dtype=np.float32)
    b1 = np.asarray(b1, dtype=np.float32)
    W2 = np.asarray(W2, dtype=np.float32)
    b2 = np.asarray(b2, dtype=np.float32)

    masked_scores, counts = routing_host(routing_scores)

    # xT in chunk-major layout [c, p, ko, t_local]
    x_tok = x.reshape(T, D).astype(ml_dtypes.bfloat16)
    xt = np.ascontiguousarray(
        x_tok.reshape(NCHUNK, TC, KO1, P).transpose(0, 3, 2, 1)
    )

    in_maps = []
    for e in range(NCORES):
        w1e = np.ascontiguousarray(
            W1[e].astype(ml_dtypes.bfloat16).reshape(KO1, P, M1, P).transpose(2, 1, 0, 3)
        )
        w2e = np.ascontiguousarray(
            W2[e].astype(ml_dtypes.bfloat16).reshape(KO2, P, M2, P).transpose(2, 1, 0, 3)
        )
        b1e = np.ascontiguousarray(b1[e].reshape(M1, P).T)
        b2e = np.ascontiguousarray(b2[e].reshape(M2, P).T)
        mse = np.ascontiguousarray(
            np.broadcast_to(
                masked_scores[:, :, e].reshape(1, T), (P, T)
            )
        ).astype(np.float32)
        in_maps.append(
            {"xt": xt, "w1": w1e, "w2": w2e, "b1": b1e, "b2": b2e, "msb": mse}
        )
    return in_maps, counts


def gather_output(results):
    outT = np.concatenate(
        [results[r]["out_shard"] for r in range(NCORES)], axis=0
    )                                                              # [D, T]
    return np.ascontiguousarray(outT.T).reshape(B, S, D)


def kernel(x, routing_scores, W1, b1, W2, b2):
    nc = build_nc()
    in_maps, counts = make_in_maps(x, routing_scores, W1, b1, W2, b2)
    res = run_bass_kernel_spmd(nc, in_maps, core_ids=list(range(NCORES)))
    final = gather_output(res.results)
    return final, counts
